# revision 1
# baseline (speedup 1.0000x reference)
"""Trainium2 Bass kernel for nn_CONV_A_64115271795341.

The module (im2col mean-centered conv + linear on window means) folds exactly
into a single 3x3 edge-padded convolution with effective weights:

  W_eff[c,k,d] = weight[c,k,d] + (w_lin[d,c] - sum_k weight[c,k,d]) / 9

Sharding: data-parallel over batch (8 images -> 8 NeuronCores), weights
replicated.

Per-core layout:
  - host pre-pads each image to [64, 130*130] (edge padding), tagged fp32r.
  - SBUF xp[128, 130*130] fp32r: partitions 0-63 = padded image, partitions
    64-127 = same data shifted by +1 element (on-chip DVE copy). A K=128
    matmul at base offset o then contracts offset j and j+1 simultaneously
    ("pair" matmuls cover kernel taps (i,0)+(i,1), i = row tap).
  - taps (i,2) run as bf16 K=64 matmuls on the idle PE column-groups
    (tile_position=(0,64)) reading a bf16 copy made by GPSIMD. (fp32r
    matmuls cannot target dst partitions 64-127; bf16 can.)
  - Per 4-row output tile: PSUM A (pairs) + PSUM B (singles) merged by
    ACT copy (PSUM->SBUF) + DVE add (PSUM+SBUF->SBUF), then HWDGE store.
"""

import numpy as np

C, H, W, D, B = 64, 128, 128, 64, 8
KS = 3
WP = W + 2            # 130
HP = H + 2
NP = WP * HP          # 16900 padded elems
TILE_ROWS = 4
NTILES = H // TILE_ROWS          # 32 tiles of [64, 512]
TN = TILE_ROWS * W               # 512
IN_CHUNKS = 8
DUP_CHUNKS = 8

_CACHE = {}


def _build():
    import concourse.bass as bass  # noqa: F401
    import concourse.mybir as mybir
    import concourse.tile as tile
    from concourse import bacc

    dt = mybir.dt
    nc = bacc.Bacc("TRN2", target_bir_lowering=False, debug=False, num_devices=8)

    x_d = nc.dram_tensor("x", [C, NP], dt.float32r, kind="ExternalInput")
    wpair_d = nc.dram_tensor("wpair", [128, 3 * D], dt.float32r, kind="ExternalInput")
    wsing_d = nc.dram_tensor("wsing", [C, 3 * D], dt.bfloat16, kind="ExternalInput")
    out_d = nc.dram_tensor("out", [D, H * W], dt.float32, kind="ExternalOutput")

    with tile.TileContext(nc) as tc:
        with tc.tile_pool(name="io", bufs=1) as io_pool, \
             tc.tile_pool(name="outp", bufs=4) as out_pool, \
             tc.tile_pool(name="tmpp", bufs=3) as tmp_pool, \
             tc.tile_pool(name="psa", bufs=3, space="PSUM") as psa_pool, \
             tc.tile_pool(name="psb", bufs=3, space="PSUM") as psb_pool:

            wpair_sb = io_pool.tile([128, 3 * D], dt.float32r, name="wpair_sb")
            nc.sync.dma_start(wpair_sb[:, :], wpair_d.ap()[:, :])
            wsing_sb = io_pool.tile([C, 3 * D], dt.bfloat16, name="wsing_sb")
            nc.sync.dma_start(wsing_sb[:, :], wsing_d.ap()[:, :])

            xp = io_pool.tile([128, NP], dt.float32r, name="xp")
            xb = io_pool.tile([C, NP], dt.bfloat16, name="xb")

            # input DMA chunks (contiguous), then shifted dup (DVE) and bf16
            # copy (GPSIMD) per chunk
            bounds = [NP * g // IN_CHUNKS for g in range(IN_CHUNKS + 1)]
            for g in range(IN_CHUNKS):
                a, b = bounds[g], bounds[g + 1]
                nc.sync.dma_start(xp[0:C, a:b], x_d.ap()[:, a:b])
            for g in range(DUP_CHUNKS):
                a, b = bounds[g], bounds[g + 1]
                be = min(b, NP - 1)
                nc.vector.tensor_copy(xp[C:128, a:be], xp[0:C, a + 1:be + 1])
                nc.gpsimd.tensor_copy(xb[:, a:b], xp[0:C, a:b].bitcast(dt.float32))

            xv = xp.rearrange("p (r c) -> p r c", c=WP)
            xbv = xb.rearrange("p (r c) -> p r c", c=WP)

            for t in range(NTILES):
                h0 = t * TILE_ROWS
                psA = psa_pool.tile([64, TN], mybir.dt.float32, name="psA")
                psB = psb_pool.tile([128, TN], mybir.dt.float32, name="psB")
                for i in range(KS):
                    nc.tensor.matmul(
                        psA[:, :],
                        lhsT=wpair_sb[:, D * i:D * (i + 1)],
                        rhs=xv[:, h0 + i:h0 + i + TILE_ROWS, 0:W],
                        start=(i == 0), stop=(i == KS - 1),
                    )
                    nc.tensor.matmul(
                        psB[64:128, :],
                        lhsT=wsing_sb[:, D * i:D * (i + 1)],
                        rhs=xbv[:, h0 + i:h0 + i + TILE_ROWS, 2:WP],
                        start=(i == 0), stop=(i == KS - 1),
                        tile_position=(0, 64),
                    )
                tmp = tmp_pool.tile([64, TN], mybir.dt.float32, name="tmp")
                nc.scalar.copy(tmp[:, :], psB[64:128, :])
                outt = out_pool.tile([64, TN], mybir.dt.float32, name="outt")
                nc.vector.tensor_add(outt[:, :], psA[:, :], tmp[:, :])
                nc.scalar.dma_start(out_d.ap()[:, TN * t:TN * (t + 1)], outt[:, :])

    nc.compile()
    return nc


def _prep_inputs(x, weight, w_lin):
    import ml_dtypes
    # effective conv weights
    w = weight.astype(np.float64)
    weff = w + (w_lin.astype(np.float64).T[:, None, :] - w.sum(axis=1, keepdims=True)) / 9.0
    weff = weff.astype(np.float32)                      # [C, 9, D]
    wpair = np.empty((128, 3 * D), np.float32)
    wsing = np.empty((C, 3 * D), np.float32)
    for i in range(KS):
        wpair[0:C, D * i:D * (i + 1)] = weff[:, 3 * i + 0, :]
        wpair[C:128, D * i:D * (i + 1)] = weff[:, 3 * i + 1, :]
        wsing[:, D * i:D * (i + 1)] = weff[:, 3 * i + 2, :]
    wsing = wsing.astype(ml_dtypes.bfloat16)

    # pre-pad each image with edge padding, flatten
    xp = np.pad(np.asarray(x), ((0, 0), (0, 0), (1, 1), (1, 1)), mode="edge")
    xp = xp.reshape(B, C, NP).astype(np.float32)
    return xp, wpair, wsing


def kernel(x, weight, w_lin):
    from concourse.bass_utils import run_bass_kernel_spmd

    if "nc" not in _CACHE:
        _CACHE["nc"] = _build()
    nc = _CACHE["nc"]

    xp, wpair, wsing = _prep_inputs(x, weight, w_lin)
    in_maps = [
        {"x": xp[b], "wpair": wpair, "wsing": wsing}
        for b in range(B)
    ]
    res = run_bass_kernel_spmd(nc, in_maps, core_ids=list(range(B)))
    out = np.stack([res.results[b]["out"].reshape(D, H, W) for b in range(B)])
    return out.astype(np.float32)


# revision 2
# speedup vs baseline: 1.6693x; 1.6693x over previous
"""Trainium2 Bass kernel for nn_CONV_A_64115271795341.

The module (im2col mean-centered conv + linear on window means) folds exactly
into a single 3x3 edge-padded convolution with effective weights:

  W_eff[c,k,d] = weight[c,k,d] + (w_lin[d,c] - sum_k weight[c,k,d]) / 9

Sharding: data-parallel over batch (8 images -> 8 NeuronCores), weights
replicated.

Per-core design:
  - host pre-pads each image to [64, 130*130] (edge padding), shipped fp32r.
  - SBUF xp[128, NP] fp32r: partitions 0-63 = padded image, partitions
    64-127 = same data shifted +1 element (DVE copy). A K=128 matmul at
    base offset o contracts taps j and j+1 at once: "pair" matmuls cover
    kernel taps (i,0)+(i,1) for each kernel row i.
  - taps (i,2) run as bf16 K=64 matmuls on PE column-groups 2-3
    (tile_position=(0,64)) reading a bf16 copy made by GPSIMD; fp32r
    matmuls cannot write dst partitions 64-127, bf16 can, and this keeps
    both halves of the PE array busy concurrently.
  - epilogue fused over pairs of output tiles: ACT copies PSUM-B to SBUF,
    DVE adds PSUM-A + SBUF, HWDGE stores [64, 1024] chunks.
"""

import numpy as np

C, H, W, D, B = 64, 128, 128, 64, 8
KS = 3
WP = W + 2            # 130
HP = H + 2
NP = WP * HP          # 16900 padded elems
TILE_ROWS = 4
NTILES = H // TILE_ROWS          # 32 tiles of [64, 512]
TN = TILE_ROWS * W               # 512
GROUP = 2                        # output tiles fused per epilogue op
NGROUPS = NTILES // GROUP
IN_CHUNKS = 8
DUP_CHUNKS = 4
CAST_CHUNKS = 8

_CACHE = {}


def _build(repeat=1):
    import concourse.bass as bass  # noqa: F401
    import concourse.mybir as mybir
    import concourse.tile as tile
    from concourse import bacc

    dt = mybir.dt
    nc = bacc.Bacc("TRN2", target_bir_lowering=False, debug=False, num_devices=8)

    x_d = nc.dram_tensor("x", [C, NP], dt.float32r, kind="ExternalInput")
    wpair_d = nc.dram_tensor("wpair", [128, 3 * D], dt.float32r, kind="ExternalInput")
    wsing_d = nc.dram_tensor("wsing", [C, 3 * D], dt.bfloat16, kind="ExternalInput")
    out_d = nc.dram_tensor("out", [D, H * W], dt.float32, kind="ExternalOutput")

    with tile.TileContext(nc) as tc:
        with tc.tile_pool(name="io", bufs=1) as io_pool, \
             tc.tile_pool(name="outp", bufs=3) as out_pool, \
             tc.tile_pool(name="tmpp", bufs=2) as tmp_pool, \
             tc.tile_pool(name="psa", bufs=2, space="PSUM") as psa_pool, \
             tc.tile_pool(name="psb", bufs=2, space="PSUM") as psb_pool:

            for _rep in range(repeat):
                wpair_sb = io_pool.tile([128, 3 * D], dt.float32r,
                                        name="wpair_sb", tag="wpair_sb")
                nc.sync.dma_start(wpair_sb[:, :], wpair_d.ap()[:, :])
                wsing_sb = io_pool.tile([C, 3 * D], dt.bfloat16,
                                        name="wsing_sb", tag="wsing_sb")
                nc.sync.dma_start(wsing_sb[:, :], wsing_d.ap()[:, :])

                xp = io_pool.tile([128, NP], dt.float32r, name="xp", tag="xp")
                xb = io_pool.tile([C, NP], dt.bfloat16, name="xb", tag="xb")

                bnd = [NP * g // IN_CHUNKS for g in range(IN_CHUNKS + 1)]
                for g in range(IN_CHUNKS):
                    a, b = bnd[g], bnd[g + 1]
                    nc.sync.dma_start(xp[0:C, a:b], x_d.ap()[:, a:b])
                dbnd = [NP * g // DUP_CHUNKS for g in range(DUP_CHUNKS + 1)]
                for g in range(DUP_CHUNKS):
                    a, b = dbnd[g], dbnd[g + 1]
                    be = min(b, NP - 1)
                    nc.vector.tensor_copy(xp[C:128, a:be], xp[0:C, a + 1:be + 1])
                cbnd = [NP * g // CAST_CHUNKS for g in range(CAST_CHUNKS + 1)]
                for g in range(CAST_CHUNKS):
                    a, b = cbnd[g], cbnd[g + 1]
                    nc.gpsimd.tensor_copy(xb[:, a:b], xp[0:C, a:b].bitcast(dt.float32))

                xv = xp.rearrange("p (r c) -> p r c", c=WP)
                xbv = xb.rearrange("p (r c) -> p r c", c=WP)

                for grp in range(NGROUPS):
                    psA = psa_pool.tile([64, GROUP * TN], mybir.dt.float32,
                                        name="psA", tag="psA")
                    psB = psb_pool.tile([128, GROUP * TN], mybir.dt.float32,
                                        name="psB", tag="psB")
                    for s in range(GROUP):
                        t = grp * GROUP + s
                        h0 = t * TILE_ROWS
                        for i in range(KS):
                            nc.tensor.matmul(
                                psA[:, TN * s:TN * (s + 1)],
                                lhsT=wpair_sb[:, D * i:D * (i + 1)],
                                rhs=xv[:, h0 + i:h0 + i + TILE_ROWS, 0:W],
                                start=(i == 0), stop=(i == KS - 1),
                            )
                            nc.tensor.matmul(
                                psB[64:128, TN * s:TN * (s + 1)],
                                lhsT=wsing_sb[:, D * i:D * (i + 1)],
                                rhs=xbv[:, h0 + i:h0 + i + TILE_ROWS, 2:WP],
                                start=(i == 0), stop=(i == KS - 1),
                                tile_position=(0, 64),
                            )
                    tmp = tmp_pool.tile([64, GROUP * TN], mybir.dt.float32,
                                        name="tmp", tag="tmp")
                    nc.scalar.copy(tmp[:, :], psB[64:128, :])
                    outt = out_pool.tile([64, GROUP * TN], mybir.dt.float32,
                                         name="outt", tag="outt")
                    nc.vector.tensor_add(outt[:, :], psA[:, :], tmp[:, :])
                    nc.scalar.dma_start(
                        out_d.ap()[:, GROUP * TN * grp:GROUP * TN * (grp + 1)],
                        outt[:, :])

    nc.compile()
    return nc


def _prep_inputs(x, weight, w_lin):
    import ml_dtypes
    w = weight.astype(np.float64)
    weff = w + (w_lin.astype(np.float64).T[:, None, :] - w.sum(axis=1, keepdims=True)) / 9.0
    weff = weff.astype(np.float32)                      # [C, 9, D]
    wpair = np.empty((128, 3 * D), np.float32)
    wsing = np.empty((C, 3 * D), np.float32)
    for i in range(KS):
        wpair[0:C, D * i:D * (i + 1)] = weff[:, 3 * i + 0, :]
        wpair[C:128, D * i:D * (i + 1)] = weff[:, 3 * i + 1, :]
        wsing[:, D * i:D * (i + 1)] = weff[:, 3 * i + 2, :]
    wsing = wsing.astype(ml_dtypes.bfloat16)

    xp = np.pad(np.asarray(x), ((0, 0), (0, 0), (1, 1), (1, 1)), mode="edge")
    xp = xp.reshape(B, C, NP).astype(np.float32)
    return xp, wpair, wsing


def kernel(x, weight, w_lin):
    from concourse.bass_utils import run_bass_kernel_spmd

    if "nc" not in _CACHE:
        _CACHE["nc"] = _build()
    nc = _CACHE["nc"]

    xp, wpair, wsing = _prep_inputs(x, weight, w_lin)
    in_maps = [
        {"x": xp[b], "wpair": wpair, "wsing": wsing}
        for b in range(B)
    ]
    res = run_bass_kernel_spmd(nc, in_maps, core_ids=list(range(B)))
    out = np.stack([res.results[b]["out"].reshape(D, H, W) for b in range(B)])
    return out.astype(np.float32)
